# revision 1
# baseline (speedup 1.0000x reference)
"""Trainium2 Bass kernel for DiffusionCoordinateInitializer.

Reference computation:
    coords = einsum("bsd,cd->bsc", latent, W) + b          # [B, S, 3]
    x = noise; for t in reversed(range(T)): x = a*x + (1-a)*coords, a=(t+1)/T
which collapses (affine fixed-point iteration) to
    x = A*noise + (1-A)*(coords + b),  A = prod_{t=1..T} t/T = T!/T^T

Strategy (pure data-parallel over 8 cores, seq-sharded):
  - Host folds (1-A) into W^T and A*noise + (1-A)*b into a bias tensor, so
    the device computes out^T[3, tok] = (Wt^T @ latent^T) + bias^T.
  - Each core streams its 4096-token shard of latent [tok, 2048] (fp32,
    memory-bound: 32 MB/core), transposes 128x128 blocks on TensorE
    (transpose mode), copies PSUM->SBUF on Vector/Scalar engines, and runs
    skinny accumulating matmuls (W^T chunk stationary [128,3], latent^T
    moving [128,512]) in fp32r for 1 cycle/row throughput.
  - Output comes back token-transposed [3, 4096] per core; host flips it.
"""

import numpy as np
from contextlib import ExitStack

import concourse.bass as bass  # noqa: F401
import concourse.tile as tile
from concourse import bacc, mybir
from concourse.bass_utils import run_bass_kernel_spmd

N_CORES = 8
B, S, D = 4, 8192, 2048
TOK = B * S                      # 32768
TPC = TOK // N_CORES             # 4096 tokens per core
P = 128
SUPER = 512                      # tokens per super-tile (matmul moving dim)
N_SUPER = TPC // SUPER           # 8
N_CHUNK = D // P                 # 16
F32 = mybir.dt.float32
F32R = mybir.dt.float32r
F16 = mybir.dt.float16

# Data-path dtype mode for the latent stream:
#   "f32r_dram": latent DRAM tensor declared float32r (same bits as fp32) so
#       plain HWDGE loads feed f32r transposes/matmuls with no cast anywhere.
#   True:  gpsimd SWDGE DMA converts fp32->f32r on load (~17% slower DMA).
#   False: HWDGE fp32 loads + fp32 transposes; copies round to f32r.
CAST_DMA = "f32r_dram"

_NC_CACHE = {}


def _build_nc(cast_dma=CAST_DMA, repeat=1, hw_loop=False):
    key = ("v1", cast_dma, repeat, hw_loop)
    if key in _NC_CACHE:
        return _NC_CACHE[key]

    nc = bacc.Bacc("TRN2", target_bir_lowering=False, debug=False,
                   enable_asserts=False, num_devices=N_CORES)
    lat_dram_dt = F32R if cast_dma == "f32r_dram" else F32
    lat = nc.dram_tensor("lat", [TPC, D], lat_dram_dt, kind="ExternalInput").ap()
    # host prepacks W^T chunks as [128, 16*3]; declared f32r so plain HWDGE
    # loads need no cast (bits are identical, fp32r is a matmul-mode tag)
    wt = nc.dram_tensor("wt", [P, 3 * N_CHUNK], F32R, kind="ExternalInput").ap()
    nzt = nc.dram_tensor("nzt", [3, TPC], F32, kind="ExternalInput").ap()
    ident = nc.dram_tensor("ident", [P, P], F32R, kind="ExternalInput").ap()
    out = nc.dram_tensor("out", [3, TPC], F32, kind="ExternalOutput").ap()

    lat_dt = F32R if cast_dma else F32  # f32r_dram is truthy -> F32R tiles

    with tile.TileContext(nc) as tc:
        with ExitStack() as ctx:
            const = ctx.enter_context(tc.tile_pool(name="const", bufs=1))
            lat_pool = ctx.enter_context(tc.tile_pool(name="lat", bufs=8))
            latT_pool = ctx.enter_context(tc.tile_pool(name="latT", bufs=2))
            ps_pool = ctx.enter_context(tc.tile_pool(name="ps", bufs=4, space="PSUM"))
            cps_pool = ctx.enter_context(tc.tile_pool(name="cps", bufs=2, space="PSUM"))
            nz_pool = ctx.enter_context(tc.tile_pool(name="nz", bufs=2))
            osb_pool = ctx.enter_context(tc.tile_pool(name="osb", bufs=3))

            # constants: identity for TensorE transpose, W^T chunks (prepacked)
            id_t = const.tile([P, P], F32R if cast_dma else F32)
            nc.sync.dma_start(id_t[:], ident[:].bitcast(id_t.dtype))
            wt_t = const.tile([P, 3 * N_CHUNK], F32R)
            nc.sync.dma_start(wt_t[:], wt[:])

            state = {"ncopy": 0}

            def super_tile(sup):
                t0 = sup * SUPER
                lats = []
                for s in range(SUPER // P):
                    lt = lat_pool.tile([P, D], lat_dt)
                    if cast_dma is True:
                        nc.gpsimd.dma_start(lt[:], lat[t0 + s * P:t0 + (s + 1) * P, :])
                    else:
                        nc.sync.dma_start(lt[:], lat[t0 + s * P:t0 + (s + 1) * P, :])
                    lats.append(lt)

                latT = latT_pool.tile([P, N_CHUNK * SUPER], F32R)
                # 4 transposes share one PSUM bank (each is a complete
                # start/stop group over a disjoint 512B quarter), then one
                # [128, 512] copy moves the whole bank to SBUF. Chunk-blocks
                # of 4 keep PSUM usage at 4 banks; the last-arriving token
                # tile's transposes are issued last so the PE doesn't stall
                # on it early and the per-super tail chain stays short.
                n_s = SUPER // P
                for kb in range(N_CHUNK // 4):
                    ks = range(4 * kb, 4 * kb + 4)
                    pss = {}
                    for k in ks:
                        pss[k] = ps_pool.tile([P, SUPER], lat_dt,
                                              name="ps", tag="ps")
                    for s in range(n_s - 1):
                        for k in ks:
                            nc.tensor.transpose(pss[k][:, s * P:(s + 1) * P],
                                                lats[s][:, k * P:(k + 1) * P],
                                                id_t[:])
                    for k in ks:
                        nc.tensor.transpose(pss[k][:, (n_s - 1) * P:n_s * P],
                                            lats[n_s - 1][:, k * P:(k + 1) * P],
                                            id_t[:])
                        dst = latT[:, k * SUPER:(k + 1) * SUPER]
                        if state["ncopy"] % 2 == 0:
                            nc.vector.tensor_copy(dst, pss[k][:])
                        else:
                            nc.scalar.copy(dst, pss[k][:])
                        state["ncopy"] += 1

                cps = cps_pool.tile([3, SUPER], F32)
                for k in range(N_CHUNK):
                    nc.tensor.matmul(
                        cps[:], wt_t[:, k * 3:(k + 1) * 3],
                        latT[:, k * SUPER:(k + 1) * SUPER],
                        start=(k == 0), stop=(k == N_CHUNK - 1),
                    )

                nz_t = nz_pool.tile([3, SUPER], F32)
                nc.sync.dma_start(nz_t[:], nzt[:, t0:t0 + SUPER])
                osb = osb_pool.tile([3, SUPER], F32)
                nc.vector.tensor_add(osb[:], cps[:], nz_t[:])
                nc.sync.dma_start(out[:, t0:t0 + SUPER], osb[:])

            if hw_loop and repeat > 1:
                with tc.For_i(0, repeat, 1):
                    for sup in range(N_SUPER):
                        super_tile(sup)
            else:
                for sup_r in range(N_SUPER * repeat):
                    super_tile(sup_r % N_SUPER)

    nc.compile()
    _NC_CACHE[key] = nc
    return nc


def _build_nc_fp16(repeat=1, hw_loop=False):
    """v5: fp16 on-chip pipeline.

    HWDGE fp32 loads (full HBM rate) -> DVE/ACT cast to fp16 -> TensorE
    transposes (fp16 FWL weight loads, ~2x cheaper than f32r) packing 8
    transposes per PSUM bank -> one [128,1024] fp16 copy per 2 chunks ->
    fp16 matmuls (W^T chunk stationary, fp32 PSUM accumulate).
    """
    key = ("v5", repeat, hw_loop)
    if key in _NC_CACHE:
        return _NC_CACHE[key]

    nc = bacc.Bacc("TRN2", target_bir_lowering=False, debug=False,
                   enable_asserts=False, num_devices=N_CORES)
    lat = nc.dram_tensor("lat", [TPC, D], F32, kind="ExternalInput").ap()
    wt = nc.dram_tensor("wt", [P, 3 * N_CHUNK], F16, kind="ExternalInput").ap()
    nzt = nc.dram_tensor("nzt", [3, TPC], F32, kind="ExternalInput").ap()
    ident = nc.dram_tensor("ident", [P, P], F16, kind="ExternalInput").ap()
    out = nc.dram_tensor("out", [3, TPC], F32, kind="ExternalOutput").ap()

    n_s = SUPER // P

    with tile.TileContext(nc) as tc:
        with ExitStack() as ctx:
            const = ctx.enter_context(tc.tile_pool(name="const", bufs=1))
            lat32_pool = ctx.enter_context(tc.tile_pool(name="lat32", bufs=12))
            lat16_pool = ctx.enter_context(tc.tile_pool(name="lat16", bufs=10))
            latT_pool = ctx.enter_context(tc.tile_pool(name="latT", bufs=2))
            ps_pool = ctx.enter_context(tc.tile_pool(name="ps", bufs=4, space="PSUM"))
            cps_pool = ctx.enter_context(tc.tile_pool(name="cps", bufs=2, space="PSUM"))
            nz_pool = ctx.enter_context(tc.tile_pool(name="nz", bufs=2))
            osb_pool = ctx.enter_context(tc.tile_pool(name="osb", bufs=3))

            id_t = const.tile([P, P], F16)
            nc.sync.dma_start(id_t[:], ident[:])
            wt_t = const.tile([P, 3 * N_CHUNK], F16)
            nc.sync.dma_start(wt_t[:], wt[:])

            state = {"n": 0}

            def super_tile(sup):
                t0 = sup * SUPER
                lt16s = []
                for s in range(n_s):
                    lt32 = lat32_pool.tile([P, D], F32, name="lt32", tag="lt32")
                    nc.sync.dma_start(lt32[:], lat[t0 + s * P:t0 + (s + 1) * P, :])
                    lt16 = lat16_pool.tile([P, D], F16, name="lt16", tag="lt16")
                    if state["n"] % 2 == 0:
                        nc.vector.tensor_copy(lt16[:], lt32[:])
                    else:
                        nc.scalar.copy(lt16[:], lt32[:])
                    state["n"] += 1
                    lt16s.append(lt16)

                latT = latT_pool.tile([P, N_CHUNK * SUPER], F16, name="latT", tag="latT")
                for kb in range(N_CHUNK // 2):
                    k0 = 2 * kb
                    ps = ps_pool.tile([P, 2 * SUPER], F16, name="ps", tag="ps")
                    for s in range(n_s):
                        for dk in range(2):
                            k = k0 + dk
                            nc.tensor.transpose(
                                ps[:, dk * SUPER + s * P: dk * SUPER + (s + 1) * P],
                                lt16s[s][:, k * P:(k + 1) * P], id_t[:])
                    dst = latT[:, k0 * SUPER:(k0 + 2) * SUPER]
                    if state["n"] % 2 == 0:
                        nc.vector.tensor_copy(dst, ps[:])
                    else:
                        nc.scalar.copy(dst, ps[:])
                    state["n"] += 1

                cps = cps_pool.tile([3, SUPER], F32, name="cps", tag="cps")
                for k in range(N_CHUNK):
                    nc.tensor.matmul(
                        cps[:], wt_t[:, k * 3:(k + 1) * 3],
                        latT[:, k * SUPER:(k + 1) * SUPER],
                        start=(k == 0), stop=(k == N_CHUNK - 1),
                    )

                nz_t = nz_pool.tile([3, SUPER], F32, name="nz_t", tag="nz")
                nc.sync.dma_start(nz_t[:], nzt[:, t0:t0 + SUPER])
                osb = osb_pool.tile([3, SUPER], F32, name="osb", tag="osb")
                nc.vector.tensor_add(osb[:], cps[:], nz_t[:])
                nc.sync.dma_start(out[:, t0:t0 + SUPER], osb[:])

            if hw_loop and repeat > 1:
                with tc.For_i(0, repeat, 1):
                    for sup in range(N_SUPER):
                        super_tile(sup)
            else:
                for sup_r in range(N_SUPER * repeat):
                    super_tile(sup_r % N_SUPER)

    nc.compile()
    _NC_CACHE[key] = nc
    return nc


def _build_nc_v6(repeat=1, hw_loop=False):
    """v6 hybrid: per 512-token super-tile, 2 token-tiles go through the
    TensorE transpose+matmul route and 2 are computed directly on VectorE
    with fused multiply-reduce (scalar_tensor_tensor, fp32-exact).

    This splits the layout-conversion burden: the PE route costs ~205 ns per
    128x128 transpose (LDWEIGHTS-bound for 4-byte dtypes), which for all 4
    tiles exceeds the ~94 us HBM streaming floor; offloading half the tokens
    to the otherwise-idle VectorE brings every engine under the DMA roof.
    """
    key = ("v6", repeat, hw_loop)
    if key in _NC_CACHE:
        return _NC_CACHE[key]

    HALF = SUPER // 2  # tokens per route per super (256)

    nc = bacc.Bacc("TRN2", target_bir_lowering=False, debug=False,
                   enable_asserts=False, num_devices=N_CORES)
    lat = nc.dram_tensor("lat", [TPC, D], F32R, kind="ExternalInput").ap()
    wt = nc.dram_tensor("wt", [P, 3 * N_CHUNK], F32R, kind="ExternalInput").ap()
    weff = nc.dram_tensor("weff", [1, 3 * D], F32R, kind="ExternalInput").ap()
    nzt = nc.dram_tensor("nzt", [3, TPC], F32, kind="ExternalInput").ap()
    nz2 = nc.dram_tensor("nz2", [TPC, 3], F32, kind="ExternalInput").ap()
    ident = nc.dram_tensor("ident", [P, P], F32R, kind="ExternalInput").ap()
    out = nc.dram_tensor("out", [3, TPC], F32, kind="ExternalOutput").ap()
    out2 = nc.dram_tensor("out2", [TPC, 3], F32, kind="ExternalOutput").ap()

    with tile.TileContext(nc) as tc:
        with ExitStack() as ctx:
            const = ctx.enter_context(tc.tile_pool(name="const", bufs=1))
            lat_pool = ctx.enter_context(tc.tile_pool(name="lat", bufs=10))
            latT_pool = ctx.enter_context(tc.tile_pool(name="latT", bufs=2))
            scr_pool = ctx.enter_context(tc.tile_pool(name="scr", bufs=3))
            acc_pool = ctx.enter_context(tc.tile_pool(name="acc", bufs=4))
            ps_pool = ctx.enter_context(tc.tile_pool(name="ps", bufs=4, space="PSUM"))
            cps_pool = ctx.enter_context(tc.tile_pool(name="cps", bufs=2, space="PSUM"))
            wbps_pool = ctx.enter_context(tc.tile_pool(name="wbps", bufs=2, space="PSUM"))
            nz_pool = ctx.enter_context(tc.tile_pool(name="nz", bufs=4))
            osb_pool = ctx.enter_context(tc.tile_pool(name="osb", bufs=4))

            id_t = const.tile([P, P], F32R)
            nc.sync.dma_start(id_t[:], ident[:])
            wt_t = const.tile([P, 3 * N_CHUNK], F32R)
            nc.sync.dma_start(wt_t[:], wt[:])

            # materialize W_eff rows broadcast across partitions [128, D] x3
            # via TensorE: ones[1,128].T @ w_row_slice[1,512]
            ones_t = const.tile([1, P], F32)
            nc.vector.memset(ones_t[:], 1.0)
            wrow = const.tile([1, 3 * D], F32R)
            nc.sync.dma_start(wrow[:], weff[:])
            wb = const.tile([P, 3 * D], F32R)
            for c in range(3):
                for j in range(D // SUPER):
                    wps = wbps_pool.tile([P, SUPER], F32, name="wps", tag="wps")
                    nc.tensor.matmul(
                        wps[:], ones_t[:],
                        wrow[:, c * D + j * SUPER:c * D + (j + 1) * SUPER].bitcast(F32),
                        start=True, stop=True)
                    nc.vector.tensor_copy(
                        wb[:, c * D + j * SUPER:c * D + (j + 1) * SUPER], wps[:])

            state = {"n": 0}

            def super_tile(sup):
                t0 = sup * SUPER
                lats = []
                for s in range(SUPER // P):
                    lt = lat_pool.tile([P, D], F32R, name="lt", tag="lt")
                    nc.sync.dma_start(lt[:], lat[t0 + s * P:t0 + (s + 1) * P, :])
                    lats.append(lt)

                # --- direct route: tiles s=0,1 on VectorE ---
                for s in range(2):
                    tt = t0 + s * P
                    acc = acc_pool.tile([P, 3], F32, name="acc", tag="acc")
                    for c in range(3):
                        sc = scr_pool.tile([P, D], F32, name="sc", tag="sc")
                        nc.vector.scalar_tensor_tensor(
                            out=sc[:], in0=lats[s][:], scalar=1.0,
                            in1=wb[:, c * D:(c + 1) * D],
                            op0=mybir.AluOpType.mult, op1=mybir.AluOpType.mult,
                            accum_out=acc[:, c:c + 1])
                    nzd = nz_pool.tile([P, 3], F32, name="nzd", tag="nzd")
                    nc.sync.dma_start(nzd[:], nz2[tt:tt + P, :])
                    od = osb_pool.tile([P, 3], F32, name="od", tag="od")
                    nc.vector.tensor_add(od[:], acc[:], nzd[:])
                    nc.sync.dma_start(out2[tt:tt + P, :], od[:])

                # --- PE route: tiles s=2,3 ---
                latT = latT_pool.tile([P, N_CHUNK * HALF], F32R,
                                      name="latT", tag="latT")
                for kb in range(N_CHUNK // 4):
                    ks = range(4 * kb, 4 * kb + 4)
                    pss = {}
                    for k in ks:
                        pss[k] = ps_pool.tile([P, HALF], F32R,
                                              name="ps", tag="ps")
                    for si, s in enumerate((2, 3)):
                        for k in ks:
                            nc.tensor.transpose(
                                pss[k][:, si * P:(si + 1) * P],
                                lats[s][:, k * P:(k + 1) * P], id_t[:])
                    for k in ks:
                        dst = latT[:, k * HALF:(k + 1) * HALF]
                        nc.scalar.copy(dst, pss[k][:])

                cps = cps_pool.tile([3, HALF], F32, name="cps", tag="cps")
                for k in range(N_CHUNK):
                    nc.tensor.matmul(
                        cps[:], wt_t[:, k * 3:(k + 1) * 3],
                        latT[:, k * HALF:(k + 1) * HALF],
                        start=(k == 0), stop=(k == N_CHUNK - 1),
                    )
                nz_t = nz_pool.tile([3, HALF], F32, name="nz_t", tag="nz")
                nc.sync.dma_start(nz_t[:], nzt[:, t0 + HALF:t0 + SUPER])
                osb = osb_pool.tile([3, HALF], F32, name="osb", tag="osb")
                nc.vector.tensor_add(osb[:], cps[:], nz_t[:])
                nc.sync.dma_start(out[:, t0 + HALF:t0 + SUPER], osb[:])

            if hw_loop and repeat > 1:
                with tc.For_i(0, repeat, 1):
                    for sup in range(N_SUPER):
                        super_tile(sup)
            else:
                for sup_r in range(N_SUPER * repeat):
                    super_tile(sup_r % N_SUPER)

    nc.compile()
    _NC_CACHE[key] = nc
    return nc


def _coeff(T: int) -> float:
    a = 1.0
    for t in range(T):
        a *= (t + 1) / T
    return a


PIPELINE = "fp16"  # "fp16" (v5) or "f32r" (v2)


def kernel(latent, W, b, noise, diffusion_steps, _trace=False, _cast_dma=CAST_DMA,
           _pipeline=None):
    T = int(diffusion_steps)
    A = _coeff(T)
    pipeline = _pipeline or PIPELINE

    lat_flat = np.ascontiguousarray(latent.reshape(TOK, D), dtype=np.float32)
    wt_eff = np.ascontiguousarray(W.T).astype(np.float32) * np.float32(1.0 - A)
    # prepack [2048, 3] -> [128, 16*3]: chunk k (rows 128k..128k+128) at cols 3k..3k+3
    wt_packed = np.ascontiguousarray(
        wt_eff.reshape(N_CHUNK, P, 3).transpose(1, 0, 2).reshape(P, 3 * N_CHUNK))
    nz_eff = (np.float32(A) * noise.reshape(TOK, 3)
              + np.float32(1.0 - A) * b[None, :].astype(np.float32))
    nz_eff_t = np.ascontiguousarray(nz_eff.T.astype(np.float32))  # [3, TOK]

    if pipeline == "fp16":
        nc = _build_nc_fp16()
        wt_packed = wt_packed.astype(np.float16)
        ident = np.eye(P, dtype=np.float16)
    elif pipeline == "v6":
        nc = _build_nc_v6()
        ident = np.eye(P, dtype=np.float32)
    else:
        nc = _build_nc(_cast_dma)
        ident = np.eye(P, dtype=np.float32)

    in_maps = []
    for c in range(N_CORES):
        im = {
            "lat": lat_flat[c * TPC:(c + 1) * TPC],
            "wt": wt_packed,
            "nzt": np.ascontiguousarray(nz_eff_t[:, c * TPC:(c + 1) * TPC]),
            "ident": ident,
        }
        if pipeline == "v6":
            im["weff"] = np.ascontiguousarray(wt_eff.T).reshape(1, 3 * D)
            im["nz2"] = np.ascontiguousarray(nz_eff[c * TPC:(c + 1) * TPC])
        in_maps.append(im)
    res = run_bass_kernel_spmd(nc, in_maps, core_ids=list(range(N_CORES)),
                               trace=_trace)
    out = np.empty((TOK, 3), dtype=np.float32)
    for c in range(N_CORES):
        o1 = res.results[c]["out"].T  # [TPC, 3], PE-route tokens
        if pipeline == "v6":
            o2 = res.results[c]["out2"]  # [TPC, 3], direct-route tokens
            half = SUPER // 2
            oc = np.empty((TPC, 3), dtype=np.float32)
            for sup in range(N_SUPER):
                t0 = sup * SUPER
                oc[t0:t0 + half] = o2[t0:t0 + half]
                oc[t0 + half:t0 + SUPER] = o1[t0 + half:t0 + SUPER]
            out[c * TPC:(c + 1) * TPC] = oc
        else:
            out[c * TPC:(c + 1) * TPC] = o1
    if _trace:
        kernel._last_results = res
    return out.reshape(B, S, 3)



# revision 4
# speedup vs baseline: 2.0418x; 2.0418x over previous
"""Trainium2 Bass kernel for DiffusionCoordinateInitializer.

Reference computation:
    coords = einsum("bsd,cd->bsc", latent, W) + b          # [B, S, 3]
    x = noise; for t in reversed(range(T)): x = a*x + (1-a)*coords, a=(t+1)/T
which collapses (affine fixed-point iteration) to
    x = A*noise + (1-A)*(coords + b),  A = prod_{t=1..T} t/T = T!/T^T

Strategy (pure data-parallel over 8 cores, token-sharded):
  - Host folds (1-A) into W^T and A*noise + (1-A)*b into a bias tensor, so
    the device computes out^T[3, tok] = (W_eff @ latent^T) + bias^T.
  - Host pre-transposes + downcasts latent to fp16 [d, tok] per core, so the
    device streams contraction-major tiles straight into accumulating
    matmuls: no on-chip transposes (v5's PE bottleneck) and half the HBM
    traffic (DMA floor ~47 us/core instead of ~94 us).
  - Chunk-major schedule: for each 128-row d-chunk, one [128, 4096] fp16
    load feeds 8 skinny matmuls (W chunk stationary [128, 3], moving
    [128, 512]) accumulating into 8 PSUM banks, one per 512-token super.
  - DVE adds the bias tensor out of PSUM; one [3, 4096] store per core.
"""

import numpy as np
from contextlib import ExitStack

import concourse.bass as bass  # noqa: F401
import concourse.tile as tile
from concourse import bacc, mybir
from concourse.bass_utils import run_bass_kernel_spmd

N_CORES = 8
B, S, D = 4, 8192, 2048
TOK = B * S                      # 32768
TPC = TOK // N_CORES             # 4096 tokens per core
P = 128
SUPER = 512                      # tokens per PSUM bank (max psum free f32)
N_SUPER = TPC // SUPER           # 8
N_CHUNK = D // P                 # 16
F32 = mybir.dt.float32
F16 = mybir.dt.float16
F8E3 = mybir.dt.float8e3

_NC_CACHE = {}


def _build_nc_v7(lat_dt=F16, repeat=1):
    """Pre-transposed stream: latT [D, TPC] (fp16 or fp8e3m4) in DRAM,
    chunk-major accumulating matmuls into 8 PSUM banks, no transposes."""
    key = ("v7", lat_dt, repeat)
    if key in _NC_CACHE:
        return _NC_CACHE[key]

    nc = bacc.Bacc("TRN2", target_bir_lowering=False, debug=False,
                   enable_asserts=False, num_devices=N_CORES)
    latT = nc.dram_tensor("latT", [D, TPC], lat_dt, kind="ExternalInput").ap()
    # host prepacks W_eff^T chunks as [128, 16*3]: wt[p, 3k+c] = W_eff[c, 128k+p]
    wt = nc.dram_tensor("wt", [P, 3 * N_CHUNK], F16, kind="ExternalInput").ap()
    nzt = nc.dram_tensor("nzt", [3, TPC], F32, kind="ExternalInput").ap()
    out = nc.dram_tensor("out", [3, TPC], F32, kind="ExternalOutput").ap()

    with tile.TileContext(nc) as tc:
        with ExitStack() as ctx:
            const = ctx.enter_context(tc.tile_pool(name="const", bufs=1))
            lat_pool = ctx.enter_context(tc.tile_pool(name="lat", bufs=4))
            ps_pool = ctx.enter_context(tc.tile_pool(name="ps", bufs=1, space="PSUM"))
            osb_pool = ctx.enter_context(tc.tile_pool(name="osb", bufs=2))

            wt_t = const.tile([P, 3 * N_CHUNK], F16)
            nc.sync.dma_start(wt_t[:], wt[:])
            nz_t = const.tile([3, TPC], F32)
            nc.sync.dma_start(nz_t[:], nzt[:])

            for _ in range(repeat):
                pss = [ps_pool.tile([3, SUPER], F32, name=f"ps{s}", tag=f"ps{s}")
                       for s in range(N_SUPER)]
                for k in range(N_CHUNK):
                    lt = lat_pool.tile([P, TPC], F16, name="lt", tag="lt")
                    nc.sync.dma_start(lt[:], latT[k * P:(k + 1) * P, :])
                    for s in range(N_SUPER):
                        nc.tensor.matmul(
                            pss[s][:], wt_t[:, k * 3:(k + 1) * 3],
                            lt[:, s * SUPER:(s + 1) * SUPER],
                            start=(k == 0), stop=(k == N_CHUNK - 1),
                        )
                osb = osb_pool.tile([3, TPC], F32, name="osb", tag="osb")
                for s in range(N_SUPER):
                    nc.vector.tensor_add(osb[:, s * SUPER:(s + 1) * SUPER],
                                         pss[s][:], nz_t[:, s * SUPER:(s + 1) * SUPER])
                nc.sync.dma_start(out[:], osb[:])

    nc.compile()
    _NC_CACHE[key] = nc
    return nc


def _coeff(T: int) -> float:
    a = 1.0
    for t in range(T):
        a *= (t + 1) / T
    return a


def kernel(latent, W, b, noise, diffusion_steps, _trace=False):
    T = int(diffusion_steps)
    A = _coeff(T)

    lat_flat = np.ascontiguousarray(latent.reshape(TOK, D), dtype=np.float32)
    lat16T = lat_flat.astype(np.float16).T  # [D, TOK] view, strided
    wt_eff = np.ascontiguousarray(W.T).astype(np.float32) * np.float32(1.0 - A)
    # prepack [2048, 3] -> [128, 16*3]: chunk k (rows 128k..128k+128) at cols 3k..3k+3
    wt_packed = np.ascontiguousarray(
        wt_eff.reshape(N_CHUNK, P, 3).transpose(1, 0, 2).reshape(P, 3 * N_CHUNK)
    ).astype(np.float16)
    nz_eff = (np.float32(A) * noise.reshape(TOK, 3)
              + np.float32(1.0 - A) * b[None, :].astype(np.float32))
    nz_eff_t = np.ascontiguousarray(nz_eff.T.astype(np.float32))  # [3, TOK]

    nc = _build_nc_v7()
    in_maps = []
    for c in range(N_CORES):
        in_maps.append({
            "latT": np.ascontiguousarray(lat16T[:, c * TPC:(c + 1) * TPC]),
            "wt": wt_packed,
            "nzt": np.ascontiguousarray(nz_eff_t[:, c * TPC:(c + 1) * TPC]),
        })
    res = run_bass_kernel_spmd(nc, in_maps, core_ids=list(range(N_CORES)),
                               trace=_trace)
    out = np.empty((TOK, 3), dtype=np.float32)
    for c in range(N_CORES):
        out[c * TPC:(c + 1) * TPC] = res.results[c]["out"].T
    if _trace:
        kernel._last_results = res
    return out.reshape(B, S, 3)


# revision 9
# speedup vs baseline: 2.2673x; 1.1105x over previous
"""Trainium2 Bass kernel for DiffusionCoordinateInitializer.

Reference computation:
    coords = einsum("bsd,cd->bsc", latent, W) + b          # [B, S, 3]
    x = noise; for t in reversed(range(T)): x = a*x + (1-a)*coords, a=(t+1)/T
which collapses (affine fixed-point iteration) to
    x = A*noise + (1-A)*(coords + b),  A = prod_{t=1..T} t/T = T!/T^T

Strategy (pure data-parallel over 8 cores, token-sharded):
  - Host folds (1-A) into W^T and A*noise + (1-A)*b into a bias tensor, so
    the device computes out^T[3, tok] = (W_eff @ latent^T) + bias^T.
  - Host pre-transposes + downcasts latent to fp16 [d, tok] per core, so the
    device streams contraction-major tiles straight into accumulating
    matmuls: no on-chip transposes (v5's PE bottleneck) and half the HBM
    traffic (DMA floor ~47 us/core instead of ~94 us).
  - Chunk-major schedule: for each 128-row d-chunk, one [128, 4096] fp16
    load feeds 8 skinny matmuls (W chunk stationary [128, 3], moving
    [128, 512]) accumulating into 8 PSUM banks, one per 512-token super.
  - DVE adds the bias tensor out of PSUM; one [3, 4096] store per core.
"""

import numpy as np
from contextlib import ExitStack

import concourse.bass as bass  # noqa: F401
import concourse.tile as tile
from concourse import bacc, mybir
from concourse.bass_utils import run_bass_kernel_spmd

N_CORES = 8
B, S, D = 4, 8192, 2048
TOK = B * S                      # 32768
TPC = TOK // N_CORES             # 4096 tokens per core
P = 128
SUPER = 512                      # tokens per PSUM bank (max psum free f32)
N_SUPER = TPC // SUPER           # 8
N_CHUNK = D // P                 # 16
F32 = mybir.dt.float32
F16 = mybir.dt.float16
F8E3 = mybir.dt.float8e3

_NC_CACHE = {}


def _build_nc_v7(lat_dt=F16, repeat=1):
    """Pre-transposed stream: latT [D, TPC] (fp16 or fp8e3m4) in DRAM,
    chunk-major accumulating matmuls into 8 PSUM banks, no transposes."""
    key = ("v7", lat_dt, repeat)
    if key in _NC_CACHE:
        return _NC_CACHE[key]

    nc = bacc.Bacc("TRN2", target_bir_lowering=False, debug=False,
                   enable_asserts=False, num_devices=N_CORES)
    latT = nc.dram_tensor("latT", [D, TPC], lat_dt, kind="ExternalInput").ap()
    # host prepacks W_eff^T chunks as [128, 16*3]: wt[p, 3k+c] = W_eff[c, 128k+p]
    wt = nc.dram_tensor("wt", [P, 3 * N_CHUNK], F16, kind="ExternalInput").ap()
    nzt = nc.dram_tensor("nzt", [3, TPC], F32, kind="ExternalInput").ap()
    out = nc.dram_tensor("out", [3, TPC], F32, kind="ExternalOutput").ap()

    with tile.TileContext(nc) as tc:
        with ExitStack() as ctx:
            const = ctx.enter_context(tc.tile_pool(name="const", bufs=1))
            lat_pool = ctx.enter_context(tc.tile_pool(name="lat", bufs=4))
            ps_pool = ctx.enter_context(tc.tile_pool(name="ps", bufs=1, space="PSUM"))
            osb_pool = ctx.enter_context(tc.tile_pool(name="osb", bufs=2))

            wt_t = const.tile([P, 3 * N_CHUNK], F16)
            nc.sync.dma_start(wt_t[:], wt[:])
            nz_t = const.tile([3, TPC], F32)
            nc.sync.dma_start(nz_t[:], nzt[:])

            for _ in range(repeat):
                pss = [ps_pool.tile([3, SUPER], F32, name=f"ps{s}", tag=f"ps{s}")
                       for s in range(N_SUPER)]
                for k in range(N_CHUNK):
                    lt = lat_pool.tile([P, TPC], F16, name="lt", tag="lt")
                    nc.sync.dma_start(lt[:], latT[k * P:(k + 1) * P, :])
                    for s in range(N_SUPER):
                        nc.tensor.matmul(
                            pss[s][:], wt_t[:, k * 3:(k + 1) * 3],
                            lt[:, s * SUPER:(s + 1) * SUPER],
                            start=(k == 0), stop=(k == N_CHUNK - 1),
                        )
                osb = osb_pool.tile([3, TPC], F32, name="osb", tag="osb")
                for s in range(N_SUPER):
                    nc.vector.tensor_add(osb[:, s * SUPER:(s + 1) * SUPER],
                                         pss[s][:], nz_t[:, s * SUPER:(s + 1) * SUPER])
                nc.sync.dma_start(out[:], osb[:])

    nc.compile()
    _NC_CACHE[key] = nc
    return nc


PIECE = 1024                     # tokens per DMA piece (2 KB/part fp16)
N_PIECE = TPC // PIECE           # 4 pieces per chunk


def _build_nc_v8(lat_dt=F16, wt_dt=F16, repeat=1):
    """Piece-granular stream + interleaved drain.

    Same math as v7 but: each 128-row d-chunk is loaded as 4 [128, 1024]
    pieces so the first matmul starts ~8 us earlier; after the last chunk,
    each super's bias-add runs on alternating Vector/Scalar engines right
    behind its stop-matmul, and its [3, 512] store issues immediately --
    the drain hides under the PE tail instead of serializing after it.
    """
    key = ("v8", lat_dt, wt_dt, repeat)
    if key in _NC_CACHE:
        return _NC_CACHE[key]

    nc = bacc.Bacc("TRN2", target_bir_lowering=False, debug=False,
                   enable_asserts=False, num_devices=N_CORES)
    latT = nc.dram_tensor("latT", [D, TPC], lat_dt, kind="ExternalInput").ap()
    wt = nc.dram_tensor("wt", [P, 3 * N_CHUNK], wt_dt, kind="ExternalInput").ap()
    nzt = nc.dram_tensor("nzt", [3, TPC], F32, kind="ExternalInput").ap()
    out = nc.dram_tensor("out", [3, TPC], F32, kind="ExternalOutput").ap()

    SPP = PIECE // SUPER  # supers per piece (2)

    with tile.TileContext(nc) as tc:
        with ExitStack() as ctx:
            const = ctx.enter_context(tc.tile_pool(name="const", bufs=1))
            lat_pool = ctx.enter_context(tc.tile_pool(name="lat", bufs=12))
            ps_pool = ctx.enter_context(tc.tile_pool(name="ps", bufs=1, space="PSUM"))
            osb_pool = ctx.enter_context(tc.tile_pool(name="osb", bufs=8))

            wt_t = const.tile([P, 3 * N_CHUNK], wt_dt)
            nc.sync.dma_start(wt_t[:], wt[:])
            nz_t = const.tile([3, TPC], F32)
            nc.sync.dma_start(nz_t[:], nzt[:])

            for _ in range(repeat):
                pss = [ps_pool.tile([3, SUPER], F32, name=f"ps{s}", tag=f"ps{s}")
                       for s in range(N_SUPER)]
                for k in range(N_CHUNK):
                    pieces = []
                    for p in range(N_PIECE):
                        lt = lat_pool.tile([P, PIECE], lat_dt, name="lt", tag="lt")
                        nc.sync.dma_start(
                            lt[:], latT[k * P:(k + 1) * P,
                                        p * PIECE:(p + 1) * PIECE])
                        pieces.append(lt)
                    for s in range(N_SUPER):
                        nc.tensor.matmul(
                            pss[s][:], wt_t[:, k * 3:(k + 1) * 3],
                            pieces[s // SPP][:, (s % SPP) * SUPER:
                                             (s % SPP + 1) * SUPER],
                            start=(k == 0), stop=(k == N_CHUNK - 1),
                        )
                        if k == N_CHUNK - 1:
                            osb = osb_pool.tile([3, SUPER], F32,
                                                name="osb", tag="osb")
                            nc.vector.tensor_add(osb[:], pss[s][:],
                                                 nz_t[:, s * SUPER:(s + 1) * SUPER])
                            nc.sync.dma_start(
                                out[:, s * SUPER:(s + 1) * SUPER], osb[:])

    nc.compile()
    _NC_CACHE[key] = nc
    return nc


def _build_nc_v9(lat_dt=F16, wt_dt=F16, repeat=1):
    """v8 + bias-add folded into the PE and stores straight from PSUM.

    The noise/bias term enters each super's accumulation group as one extra
    matmul: stationary = I3 [3, 3], moving = nz16 [3, 512] fp16, so
    psum += I3^T @ nz = nz elementwise. No Vector/Scalar engine work at
    all; each super's [3, 512] result DMAs from PSUM as soon as its group
    stops, hiding the whole drain under the PE tail.
    """
    key = ("v9", lat_dt, wt_dt, repeat)
    if key in _NC_CACHE:
        return _NC_CACHE[key]

    nc = bacc.Bacc("TRN2", target_bir_lowering=False, debug=False,
                   enable_asserts=False, num_devices=N_CORES)
    latT = nc.dram_tensor("latT", [D, TPC], lat_dt, kind="ExternalInput").ap()
    wt = nc.dram_tensor("wt", [P, 3 * N_CHUNK], wt_dt, kind="ExternalInput").ap()
    ident3 = nc.dram_tensor("ident3", [3, 3], F16, kind="ExternalInput").ap()
    nzt = nc.dram_tensor("nzt", [3, TPC], F16, kind="ExternalInput").ap()
    out = nc.dram_tensor("out", [3, TPC], F32, kind="ExternalOutput").ap()

    SPP = PIECE // SUPER  # supers per piece (2)

    with tile.TileContext(nc) as tc:
        with ExitStack() as ctx:
            const = ctx.enter_context(tc.tile_pool(name="const", bufs=1))
            lat_pool = ctx.enter_context(tc.tile_pool(name="lat", bufs=12))
            ps_pool = ctx.enter_context(tc.tile_pool(name="ps", bufs=1, space="PSUM"))

            wt_t = const.tile([P, 3 * N_CHUNK], wt_dt)
            nc.sync.dma_start(wt_t[:], wt[:])
            id3_t = const.tile([3, 3], F16)
            nc.sync.dma_start(id3_t[:], ident3[:])
            nz_t = const.tile([3, TPC], F16)
            nc.sync.dma_start(nz_t[:], nzt[:])

            for _ in range(repeat):
                pss = [ps_pool.tile([3, SUPER], F32, name=f"ps{s}", tag=f"ps{s}")
                       for s in range(N_SUPER)]
                for k in range(N_CHUNK):
                    pieces = []
                    for p in range(N_PIECE):
                        lt = lat_pool.tile([P, PIECE], lat_dt, name="lt", tag="lt")
                        nc.sync.dma_start(
                            lt[:], latT[k * P:(k + 1) * P,
                                        p * PIECE:(p + 1) * PIECE])
                        pieces.append(lt)
                    for s in range(N_SUPER):
                        nc.tensor.matmul(
                            pss[s][:], wt_t[:, k * 3:(k + 1) * 3],
                            pieces[s // SPP][:, (s % SPP) * SUPER:
                                             (s % SPP + 1) * SUPER],
                            start=(k == 0), stop=False,
                        )
                        if k == N_CHUNK - 1:
                            nc.tensor.matmul(
                                pss[s][:], id3_t[:],
                                nz_t[:, s * SUPER:(s + 1) * SUPER],
                                start=False, stop=True,
                            )
                            nc.sync.dma_start(
                                out[:, s * SUPER:(s + 1) * SUPER], pss[s][:])

    nc.compile()
    _NC_CACHE[key] = nc
    return nc


def _coeff(T: int) -> float:
    a = 1.0
    for t in range(T):
        a *= (t + 1) / T
    return a


PIPELINE = "v8_fp8"  # "v7" | "v8_fp16" | "v8_fp8" | "v9_fp16" | "v9_fp8"


def kernel(latent, W, b, noise, diffusion_steps, _trace=False, _pipeline=None):
    import ml_dtypes
    T = int(diffusion_steps)
    A = _coeff(T)
    pipeline = _pipeline or PIPELINE
    fp8 = pipeline.endswith("fp8")
    v9 = pipeline.startswith("v9")

    lat_flat = np.ascontiguousarray(latent.reshape(TOK, D), dtype=np.float32)
    if fp8:
        latT_h = lat_flat.astype(ml_dtypes.float8_e3m4).T  # [D, TOK] view
    else:
        latT_h = lat_flat.astype(np.float16).T
    wt_eff = np.ascontiguousarray(W.T).astype(np.float32) * np.float32(1.0 - A)
    # prepack [2048, 3] -> [128, 16*3]: chunk k (rows 128k..128k+128) at cols 3k..3k+3
    wt_packed = np.ascontiguousarray(
        wt_eff.reshape(N_CHUNK, P, 3).transpose(1, 0, 2).reshape(P, 3 * N_CHUNK)
    ).astype(np.float16)
    nz_eff = (np.float32(A) * noise.reshape(TOK, 3)
              + np.float32(1.0 - A) * b[None, :].astype(np.float32))
    nz_dt = np.float16 if v9 else np.float32
    nz_eff_t = np.ascontiguousarray(nz_eff.T.astype(nz_dt))  # [3, TOK]

    lat_dt = mybir.dt.float8e3 if fp8 else F16
    if pipeline == "v7":
        nc = _build_nc_v7()
    elif v9:
        nc = _build_nc_v9(lat_dt=lat_dt)
    else:
        nc = _build_nc_v8(lat_dt=lat_dt)
    in_maps = []
    for c in range(N_CORES):
        im = {
            "latT": np.ascontiguousarray(latT_h[:, c * TPC:(c + 1) * TPC]),
            "wt": wt_packed,
            "nzt": np.ascontiguousarray(nz_eff_t[:, c * TPC:(c + 1) * TPC]),
        }
        if v9:
            im["ident3"] = np.eye(3, dtype=np.float16)
        in_maps.append(im)
    res = run_bass_kernel_spmd(nc, in_maps, core_ids=list(range(N_CORES)),
                               trace=_trace)
    out = np.empty((TOK, 3), dtype=np.float32)
    for c in range(N_CORES):
        out[c * TPC:(c + 1) * TPC] = res.results[c]["out"].T
    if _trace:
        kernel._last_results = res
    return out.reshape(B, S, 3)
